# revision 40
# baseline (speedup 1.0000x reference)
"""Trainium2 Bass kernel: ResNet BasicBlock (conv3x3-BN-ReLU-mask-conv3x3-mask-BN-residual-ReLU).

Problem shape: x[4096, 64, 7, 7], both convs 64->64 3x3 pad 1.

Strategy (pure data parallel, 8 cores, 512 images/core):
  * Channels live on SBUF partitions. Two 64-channel image streams are
    stacked into the 128 partitions ("half0" -> partitions 0-63,
    "half1" -> 64-127) so elementwise engines run at full width.
  * UNPADDED tiles: the valid-window taps never read outside the 7x7
    image (for tap (dh,dw) the input rows ly+dh-1:hy+dh-1 stay within
    [0,7)), so no zero border is needed at all. x arrives raw 7x7 bf16
    (40% fewer input bytes than a 9x9 pad layout), y1 tiles are written
    in full by the conv1 epilogue (no border memsets anywhere).
  * A 3x3 conv is 9 shifted 64x64 matmuls accumulated in PSUM; the
    128x128 PE array is split into 4 64x64 quadrants via the matmul
    base partitions (rhs base -> row group, psum base -> column group);
    four independent tap-chains (2 pairs x 2 halves) keep the array fed.
  * Valid-window taps: each non-center tap only streams the output
    pixels whose input window is inside the image (edge taps 7x6,
    corners 6x6). The center tap covers all 49 pixels and goes first
    with start=True so every psum element is initialized; the rest
    accumulate into sub-windows. This skips the ~18% of matmul rows
    that would only add zero-padding terms (bit-identical output).
  * BN scales are folded into the conv weights on the host; BN shifts are
    per-partition bias operands of the ACT/DVE epilogues.
  * Output returns as bf16 (host converts to f32): halves the out DMA.
  * Startup: one dma_start spreads its 128 per-partition descriptors
    across all 16 sync-HWDGE queues, so w1/w2 ride the sync ring right
    behind quad 0's x (landing ~0.5us after their kick) instead of the
    SWDGE path whose data starves behind the HWDGE flood and whose
    completion semaphore lags ~2.8us. Sync kick order: x_q0, w1, x_q1,
    w2, x_q2, x_q3, then steady 3-deep prefetch. The tiny BN-shift
    blob rides the otherwise-idle scalar HWDGE ring, followed by the
    ACT-table prewarm and the (late-needed) critic masks.
  * The critic masks only touch batch element 0: the host swaps it into
    quad MQ (mid-kernel), so the mask DMA and the two mask multiplies
    sit far off both the startup and drain critical paths. Cores 1-7
    run the same multiplies with all-ones masks.
  * conv2 accumulates into per-pair PSUM tiles: Tile tracks dependencies
    per-tile, so a shared tile would falsely serialize the two pairs'
    epilogues. Epilogue ALU work runs on DVE (ACT stays dedicated to the
    conv1 relu+bias, which gates the next conv2) except the final quad,
    where pair 0 races ahead (DVE -> sync DMA kick) while pair 1
    finishes on DVE/ACT and follows on the same ring.
  * Quad schedule ramps: small first quad (n=6) so the first matmul
    starts as soon as one small DMA lands, and an [8, 4] tail so the
    final conv2+epilogue+DMA drain is short without adding a quad
    round (each extra quad costs ~72 matmul issue overheads).
"""

import ml_dtypes
import numpy as np

import concourse.bass as bass  # noqa: F401  (engine namespaces live on the nc object)
import concourse.tile as tile
from concourse import bacc, mybir
from concourse.bass_utils import run_bass_kernel_spmd

F32 = mybir.dt.float32
BF16 = mybir.dt.bfloat16
NP_BF16 = ml_dtypes.bfloat16
EPS = 1e-5
B, C, H, W = 4096, 64, 7, 7
NCORES = 8
BPC = B // NCORES          # 512 images per core
# Pad-tile ring depths. xts: at round v the quad v+3 input DMA is
# emitted before conv2(v-1)'s residual read, so slots (v+3)%XBUF and
# (v-1)%XBUF must differ or the DMA is ordered before the read (4 would
# alias; 5 gives 3-deep prefetch). y1s: written by conv1(v), read by
# conv2(v) during round v+1, rewritten by conv1(v+3) - 3 suffices.
XBUF = 5
YBUF = 3

# Quad sizes: per quad, 4 chains of n images (2 pairs x 2 halves = 4n
# images, 2n slots). Ramped: small first quad (fast start) and an
# [8, 4] tail (fast drain); sum of n = 128 (512 images / 4).
QSIZES = [8] + [10] * 11 + [6, 4]
assert sum(QSIZES) == 128
QUADS = []
_sb = 0
for _n in QSIZES:
    QUADS.append((_sb, _n))
    _sb += 2 * _n
SLOTS = _sb                # 256 slots (2 per quad-chain-image)
NMAX = max(QSIZES)
MQ = 7                     # quad holding the masked batch element
# batch position (on core 0) that lands at (quad MQ, pair 0, half 0, i 0)
MSWAP = 4 * sum(QSIZES[:MQ])

# (pair_in_quad, half, colgroup): the 4 concurrent chains of a quad.
# Even pair writes PSUM naturally, odd pair swapped - this alternation is
# what keeps all four PE quadrants busy across consecutive chains.
CHAINS = [(0, 0, 0), (1, 1, 0), (0, 1, 1), (1, 0, 1)]

_CACHE = {}


# Tap emission order: center tap first. It is the only tap whose valid
# output window covers all 49 pixels, so its start=True matmul
# initializes every psum element; the other 8 taps accumulate into
# restricted windows (edge taps 7x6, corners 6x6).
TAPS = [(1, 1)] + [(dh, dw) for dh in range(3) for dw in range(3)
                   if (dh, dw) != (1, 1)]


def _emit_conv(nc, src, wview, pss, n):
    """One quad of one conv: 4 concurrent 9-tap chains (36 matmuls).
    src is an UNPADDED [128, slots, 7, 7] tile; for tap (dh,dw) the
    output window rows [ly,hy) read input rows [ly+dh-1, hy+dh-1),
    which stays inside [0,7) for every tap (same for columns).
    wview(t, half) -> [64, 64] lhsT for tap t.
    pss = per-pair [128, NMAX, 7, 7] psum banks."""
    for ti, (dh, dw) in enumerate(TAPS):
        ly, hy = max(0, 1 - dh), H - max(0, dh - 1)
        lx, hx = max(0, 1 - dw), W - max(0, dw - 1)
        t = dh * 3 + dw
        for (j, half, cg) in CHAINS:
            rhs = src[64 * half:64 * half + 64, n * j:n * (j + 1),
                      ly + dh - 1:hy + dh - 1, lx + dw - 1:hx + dw - 1]
            out = pss[j][64 * cg:64 * cg + 64, 0:n, ly:hy, lx:hx]
            nc.tensor.matmul(out, wview(t, half), rhs,
                             start=(ti == 0), stop=(ti == 8),
                             skip_group_check=True)


def _build():
    nc = bacc.Bacc("TRN2", target_bir_lowering=False, debug=False,
                   num_devices=NCORES)
    x_d = nc.dram_tensor("x", [128, SLOTS, H, W], BF16, kind="ExternalInput")
    # w1 splits into the start-critical center tap (the first matmul of
    # every chain) and the other 8 taps, so the first matmul's weight DMA
    # is 1KB/queue instead of 9KB/queue.
    w1c_d = nc.dram_tensor("w1c", [128, 64], BF16, kind="ExternalInput")
    w1r_d = nc.dram_tensor("w1r", [128, 8, 64], BF16, kind="ExternalInput")
    w2_d = nc.dram_tensor("w2", [128, 9, 64], BF16, kind="ExternalInput")
    # cs = BN shifts (2 columns, needed by the first conv1 epilogue) and
    # mk = the two critic masks (only needed at quad MQ) ride the scalar
    # HWDGE ring as two separate DMAs so the tiny start-critical piece
    # lands first.
    cs_d = nc.dram_tensor("cs", [128, 2], F32, kind="ExternalInput")
    mk_d = nc.dram_tensor("mk", [64, 98], F32, kind="ExternalInput")
    o_d = nc.dram_tensor("o", [128, SLOTS, H, W], BF16, kind="ExternalOutput")

    with tile.TileContext(nc) as tc:
        with (
            tc.tile_pool(name="singles", bufs=1) as singles,
            tc.tile_pool(name="outp", bufs=4) as out_pool,
            tc.tile_pool(name="xys", bufs=1) as xy_pool,
            tc.tile_pool(name="ps1", bufs=2, space="PSUM") as ps1_pool,
            tc.tile_pool(name="ps2", bufs=2, space="PSUM") as ps2_pool,
        ):
            w1c_sb = singles.tile([128, 64], BF16, name="w1c_sb")
            w1r_sb = singles.tile([128, 8, 64], BF16, name="w1r_sb")
            w2_sb = singles.tile([128, 9, 64], BF16, name="w2_sb")
            cs_sb = singles.tile([128, 2], F32, name="cs_sb")
            mk_sb = singles.tile([128, 98], F32, name="mk_sb")
            warm_sb = singles.tile([128, 1], F32, name="warm_sb")

            def shift(i):           # [128, 1] bias column for bn shift i
                return cs_sb[:, i:i + 1]

            def mask(k):            # [64, 7, 7] critic mask k
                return mk_sb[0:64, 49 * k:49 * (k + 1)].rearrange(
                    "p (h w) -> p h w", h=H, w=W)

            def w1view(t, half):
                if t == 4:
                    return w1c_sb[64 * half:64 * half + 64, :]
                r = t if t < 4 else t - 1
                return w1r_sb[64 * half:64 * half + 64, r, :]

            def w2view(t, half):
                return w2_sb[64 * half:64 * half + 64, t, :]

            xts = [xy_pool.tile([128, 2 * NMAX, H, W], BF16,
                                name=f"xt{i}", tag=f"xt{i}")
                   for i in range(XBUF)]
            y1s = [xy_pool.tile([128, 2 * NMAX, H, W], BF16,
                                name=f"y1{i}", tag=f"y1{i}")
                   for i in range(YBUF)]

            def emit_in_dma(v):
                base, n = QUADS[v]
                nc.sync.dma_start(xts[v % XBUF][:, 0:2 * n],
                                  x_d[:, base:base + 2 * n])

            def emit_conv2(state):
                v, base, n, yp, xp = state
                # separate per-pair psum tiles: Tile deps are per-tile, so
                # a shared tile would serialize pair 0's epilogue writes
                # behind pair 1's reads (and vice versa)
                ps2 = [ps2_pool.tile([128, NMAX, H, W], F32, name=f"ps2{j}")
                       for j in range(2)]
                _emit_conv(nc, yp, w2view, ps2, n)
                if v == MQ:
                    # critic mask 2 on conv2 output of the masked image
                    tgt = ps2[0][0:64, 0, :, :]
                    nc.vector.tensor_mul(tgt, tgt, mask(1))
                # residual adds from the bf16 x tile, back-to-back on
                # DVE; the two relu(psum+shift2) finals split DVE/ACT in
                # the last quad so they run concurrently
                views = [ps2[j][:, 0:n] for j in range(2)]
                out_q = out_pool.tile([128, 2 * NMAX, H, W], BF16,
                                      name="out_q")

                def add(j):
                    nc.vector.tensor_add(
                        views[j], views[j], xp[:, n * j:n * (j + 1)])

                def relu_p1():
                    nc.scalar.activation(
                        out=out_q[:, n:2 * n], in_=views[1],
                        func=mybir.ActivationFunctionType.Relu,
                        bias=shift(1), scale=1.0)

                def ts(j):
                    nc.vector.tensor_scalar(
                        out_q[:, n * j:n * (j + 1)], views[j],
                        shift(1), 0.0,
                        mybir.AluOpType.add, mybir.AluOpType.max)

                if v >= len(QUADS) - 2:
                    # last two quads: everything after the last matmul is
                    # pure drain. Residual adds must run on DVE (Pool
                    # cannot access PSUM), but pair 1's relu+shift moves
                    # to ACT so the two pairs' conversions overlap. Each
                    # quad ships both pairs in ONE kick: a second kick
                    # costs ~0.6us of Sync-sequencer serialization that
                    # delays the final descriptors more than the earlier
                    # pair-0 start saves (measured: a split penult kick
                    # pushed the final quad's kick ~0.9us later).
                    add(0)
                    add(1)
                    ts(0)
                    relu_p1()
                    nc.sync.dma_start(o_d[:, base:base + 2 * n],
                                      out_q[:, 0:2 * n])
                else:
                    # steady state: whole epilogue on DVE (it has slack),
                    # keeping ACT free for the conv1 RELUs so the next
                    # quad's conv2 is never gated on a queued-up ACT
                    add(0)
                    add(1)
                    ts(0)
                    ts(1)
                    nc.sync.dma_start(o_d[:, base:base + 2 * n],
                                      out_q[:, 0:2 * n])

            pending = None
            for v, (base, n) in enumerate(QUADS):
                if v == 0:
                    # prologue: one dma_start spreads over all 16 sync
                    # queues, so serial kick order is the lever: quad0's
                    # x first (gates the first matmul), then w1, then the
                    # next quad, then w2 (needed one round later). The
                    # scalar ring carries the shift blob + ACT prewarm +
                    # masks in parallel with the sync kicks.
                    emit_in_dma(0)
                    nc.sync.dma_start(w1c_sb[:], w1c_d[:])
                    nc.scalar.dma_start(cs_sb[:], cs_d[:])
                    nc.scalar.memzero(warm_sb[:])
                    nc.sync.dma_start(w1r_sb[:], w1r_d[:])
                    emit_in_dma(1)
                    nc.sync.dma_start(w2_sb[:], w2_d[:])
                    nc.scalar.dma_start(mk_sb[0:64, :], mk_d[:])
                    emit_in_dma(2)
                    emit_in_dma(3)
                elif v + 3 < len(QUADS):
                    emit_in_dma(v + 3)
                xp = xts[v % XBUF]
                ps1 = [ps1_pool.tile([128, NMAX, H, W], F32, name=f"ps1{j}")
                       for j in range(2)]
                _emit_conv(nc, xp, w1view, ps1, n)
                yp = y1s[v % YBUF]
                for j in range(2):
                    nc.scalar.activation(
                        out=yp[:, n * j:n * (j + 1)],
                        in_=ps1[j][:, 0:n],
                        func=mybir.ActivationFunctionType.Relu,
                        bias=shift(0), scale=1.0)
                if v == MQ:
                    # critic mask 1 on relu(bn1(conv1)) of the masked image
                    tgt = yp[0:64, 0, :, :]
                    nc.vector.tensor_mul(tgt, tgt, mask(0))
                if pending is not None:
                    emit_conv2(pending)
                pending = (v, base, n, yp, xp)
            emit_conv2(pending)

    nc.compile()
    return nc


def _get_nc():
    if "nc" not in _CACHE:
        _CACHE["nc"] = _build()
    return _CACHE["nc"]


def _host_pack(x, w1, g1, b1, m1, v1, w2, g2, b2, m2, v2, mask1, mask2):
    x = np.asarray(x, np.float32)
    scale1 = np.asarray(g1, np.float32) / np.sqrt(np.asarray(v1, np.float32) + EPS)
    shift1 = np.asarray(b1, np.float32) - np.asarray(m1, np.float32) * scale1
    scale2 = np.asarray(g2, np.float32) / np.sqrt(np.asarray(v2, np.float32) + EPS)
    shift2 = np.asarray(b2, np.float32) - np.asarray(m2, np.float32) * scale2

    def pack_w(w, scale):
        ws = np.asarray(w, np.float32) * scale[:, None, None, None]
        # [co, ci, kh, kw] -> [ci, tap, co], duplicated into both halves
        lhsT = ws.transpose(1, 2, 3, 0).reshape(64, 9, 64)
        return np.ascontiguousarray(np.tile(lhsT, (2, 1, 1)).astype(NP_BF16))

    wdev1, wdev2 = pack_w(w1, scale1), pack_w(w2, scale2)
    w1c = np.ascontiguousarray(wdev1[:, 4, :])
    w1r = np.ascontiguousarray(
        wdev1[:, [0, 1, 2, 3, 5, 6, 7, 8], :])
    cs = np.ascontiguousarray(
        np.tile(np.stack([shift1, shift2], 1), (2, 1)).astype(np.float32))

    def pack_mk(msk1, msk2):
        mk = np.empty((64, 98), dtype=np.float32)
        mk[:, 0:49] = np.asarray(msk1, np.float32).reshape(64, 49)
        mk[:, 49:98] = np.asarray(msk2, np.float32).reshape(64, 49)
        return np.ascontiguousarray(mk)

    # Raw (unpadded) bf16 input: [core, 128, slot, 7, 7]. Quad q (slot
    # base b, size n) holds images [g0, g0+4n) of its core: pair-major,
    # then half, then index -> partition half h holds channel block,
    # slot b + j*n + i. On core 0, batch element 0 (the masked image) is
    # swapped with position MSWAP so it lands at quad MQ's first slot.
    xb = x.reshape(NCORES, BPC, C, H, W)
    xb0 = xb[0].copy()
    xb0[[0, MSWAP]] = xb0[[MSWAP, 0]]
    xb = np.concatenate([xb0[None], xb[1:]], axis=0).astype(NP_BF16)
    xdev = np.empty((NCORES, 128, SLOTS, H, W), dtype=NP_BF16)
    g0 = 0
    for (sb, n) in QUADS:
        xq = xb[:, g0:g0 + 4 * n].reshape(NCORES, 2, 2, n, C, H, W)
        # [core, pair, half, i, c, h, w] -> [core, half, c, pair, i, h, w]
        xq = xq.transpose(0, 2, 4, 1, 3, 5, 6).reshape(
            NCORES, 128, 2 * n, H, W)
        xdev[:, :, sb:sb + 2 * n] = xq
        g0 += 4 * n

    mk0 = pack_mk(mask1, mask2)
    mk1 = np.ones((64, 98), dtype=np.float32)

    in_maps = []
    for c in range(NCORES):
        in_maps.append({
            "x": np.ascontiguousarray(xdev[c]),
            "w1c": w1c,
            "w1r": w1r,
            "w2": wdev2,
            "cs": cs,
            "mk": mk0 if c == 0 else mk1,
        })
    return in_maps


def _host_unpack(results):
    o = np.stack([results[c]["o"] for c in range(NCORES)]).astype(np.float32)
    out = np.empty((NCORES, BPC, C, H, W), dtype=np.float32)
    g0 = 0
    for (sb, n) in QUADS:
        oq = o[:, :, sb:sb + 2 * n].reshape(NCORES, 2, C, 2, n, H, W)
        # [core, half, c, pair, i, h, w] -> [core, pair, half, i, c, h, w]
        out[:, g0:g0 + 4 * n] = oq.transpose(0, 3, 1, 4, 2, 5, 6).reshape(
            NCORES, 4 * n, C, H, W)
        g0 += 4 * n
    out[0][[0, MSWAP]] = out[0][[MSWAP, 0]]
    return np.ascontiguousarray(out.reshape(B, C, H, W))


def run(trace=False, **inputs):
    nc = _get_nc()
    in_maps = _host_pack(**inputs)
    res = run_bass_kernel_spmd(nc, in_maps, core_ids=list(range(NCORES)),
                               trace=trace)
    return _host_unpack(res.results), res


def kernel(**inputs) -> np.ndarray:
    out, _ = run(trace=False, **inputs)
    return out


# revision 41
# speedup vs baseline: 1.0230x; 1.0230x over previous
"""Trainium2 Bass kernel: ResNet BasicBlock (conv3x3-BN-ReLU-mask-conv3x3-mask-BN-residual-ReLU).

Problem shape: x[4096, 64, 7, 7], both convs 64->64 3x3 pad 1.

Strategy (pure data parallel, 8 cores, 512 images/core):
  * Channels live on SBUF partitions. Two 64-channel image streams are
    stacked into the 128 partitions ("half0" -> partitions 0-63,
    "half1" -> 64-127) so elementwise engines run at full width.
  * UNPADDED tiles: the valid-window taps never read outside the 7x7
    image (for tap (dh,dw) the input rows ly+dh-1:hy+dh-1 stay within
    [0,7)), so no zero border is needed at all. x arrives raw 7x7 bf16
    (40% fewer input bytes than a 9x9 pad layout), y1 tiles are written
    in full by the conv1 epilogue (no border memsets anywhere).
  * A 3x3 conv is 9 shifted 64x64 matmuls accumulated in PSUM; the
    128x128 PE array is split into 4 64x64 quadrants via the matmul
    base partitions (rhs base -> row group, psum base -> column group);
    four independent tap-chains (2 pairs x 2 halves) keep the array fed.
  * Valid-window taps: each non-center tap only streams the output
    pixels whose input window is inside the image (edge taps 7x6,
    corners 6x6). The center tap covers all 49 pixels and goes first
    with start=True so every psum element is initialized; the rest
    accumulate into sub-windows. This skips the ~18% of matmul rows
    that would only add zero-padding terms (bit-identical output).
  * BN scales are folded into the conv weights on the host; BN shifts are
    per-partition bias operands of the ACT/DVE epilogues.
  * Output returns as bf16 (host converts to f32): halves the out DMA.
  * Startup: one dma_start spreads its 128 per-partition descriptors
    across all 16 sync-HWDGE queues, so w1/w2 ride the sync ring right
    behind quad 0's x (landing ~0.5us after their kick) instead of the
    SWDGE path whose data starves behind the HWDGE flood and whose
    completion semaphore lags ~2.8us. Sync kick order: x_q0, w1, x_q1,
    w2, x_q2, x_q3, then steady 3-deep prefetch. The tiny BN-shift
    blob rides the otherwise-idle scalar HWDGE ring, followed by the
    ACT-table prewarm and the (late-needed) critic masks.
  * The critic masks only touch batch element 0: the host swaps it into
    quad MQ (mid-kernel), so the mask DMA and the two mask multiplies
    sit far off both the startup and drain critical paths. Cores 1-7
    run the same multiplies with all-ones masks.
  * conv2 accumulates into per-pair PSUM tiles: Tile tracks dependencies
    per-tile, so a shared tile would falsely serialize the two pairs'
    epilogues. Epilogue ALU work runs on DVE (ACT stays dedicated to the
    conv1 relu+bias, which gates the next conv2) except the final quad,
    where pair 0 races ahead (DVE -> sync DMA kick) while pair 1
    finishes on DVE/ACT and follows on the same ring.
  * Quad schedule ramps: small first quad (n=6) so the first matmul
    starts as soon as one small DMA lands, and an [8, 4] tail so the
    final conv2+epilogue+DMA drain is short without adding a quad
    round (each extra quad costs ~72 matmul issue overheads).
"""

import ml_dtypes
import numpy as np

import concourse.bass as bass  # noqa: F401  (engine namespaces live on the nc object)
import concourse.tile as tile
from concourse import bacc, mybir
from concourse.bass_utils import run_bass_kernel_spmd

F32 = mybir.dt.float32
BF16 = mybir.dt.bfloat16
NP_BF16 = ml_dtypes.bfloat16
EPS = 1e-5
B, C, H, W = 4096, 64, 7, 7
NCORES = 8
BPC = B // NCORES          # 512 images per core
# Pad-tile ring depths. xts: at round v the quad v+3 input DMA is
# emitted before conv2(v-1)'s residual read, so slots (v+3)%XBUF and
# (v-1)%XBUF must differ or the DMA is ordered before the read (4 would
# alias; 5 gives 3-deep prefetch). y1s: written by conv1(v), read by
# conv2(v) during round v+1, rewritten by conv1(v+3) - 3 suffices.
XBUF = 5
YBUF = 3

# Quad sizes: per quad, 4 chains of n images (2 pairs x 2 halves = 4n
# images, 2n slots). Ramped: small first quad (fast start) and an
# [8, 4] tail (fast drain); sum of n = 128 (512 images / 4).
QSIZES = [8] + [10] * 11 + [6, 4]
assert sum(QSIZES) == 128
QUADS = []
_sb = 0
for _n in QSIZES:
    QUADS.append((_sb, _n))
    _sb += 2 * _n
SLOTS = _sb                # 256 slots (2 per quad-chain-image)
NMAX = max(QSIZES)
MQ = 7                     # quad holding the masked batch element
# batch position (on core 0) that lands at (quad MQ, pair 0, half 0, i 0)
MSWAP = 4 * sum(QSIZES[:MQ])

# (pair_in_quad, half, colgroup): the 4 concurrent chains of a quad.
# Even pair writes PSUM naturally, odd pair swapped - this alternation is
# what keeps all four PE quadrants busy across consecutive chains.
CHAINS = [(0, 0, 0), (1, 1, 0), (0, 1, 1), (1, 0, 1)]

_CACHE = {}


# Tap emission order: center tap first. It is the only tap whose valid
# output window covers all 49 pixels, so its start=True matmul
# initializes every psum element; the other 8 taps accumulate into
# restricted windows (edge taps 7x6, corners 6x6).
TAPS = [(1, 1)] + [(dh, dw) for dh in range(3) for dw in range(3)
                   if (dh, dw) != (1, 1)]


def _emit_conv(nc, src, wview, pss, n):
    """One quad of one conv: 4 concurrent 9-tap chains (36 matmuls).
    src is an UNPADDED [128, slots, 7, 7] tile; for tap (dh,dw) the
    output window rows [ly,hy) read input rows [ly+dh-1, hy+dh-1),
    which stays inside [0,7) for every tap (same for columns).
    wview(t, half) -> [64, 64] lhsT for tap t.
    pss = per-pair [128, NMAX, 7, 7] psum banks."""
    for ti, (dh, dw) in enumerate(TAPS):
        ly, hy = max(0, 1 - dh), H - max(0, dh - 1)
        lx, hx = max(0, 1 - dw), W - max(0, dw - 1)
        t = dh * 3 + dw
        for (j, half, cg) in CHAINS:
            rhs = src[64 * half:64 * half + 64, n * j:n * (j + 1),
                      ly + dh - 1:hy + dh - 1, lx + dw - 1:hx + dw - 1]
            out = pss[j][64 * cg:64 * cg + 64, 0:n, ly:hy, lx:hx]
            nc.tensor.matmul(out, wview(t, half), rhs,
                             start=(ti == 0), stop=(ti == 8),
                             skip_group_check=True)


def _build():
    nc = bacc.Bacc("TRN2", target_bir_lowering=False, debug=False,
                   num_devices=NCORES)
    x_d = nc.dram_tensor("x", [128, SLOTS, H, W], BF16, kind="ExternalInput")
    # w1 splits into the start-critical center tap (the first matmul of
    # every chain) and the other 8 taps, so the first matmul's weight DMA
    # is 1KB/queue instead of 9KB/queue.
    w1c_d = nc.dram_tensor("w1c", [128, 64], BF16, kind="ExternalInput")
    w1r_d = nc.dram_tensor("w1r", [128, 8, 64], BF16, kind="ExternalInput")
    w2_d = nc.dram_tensor("w2", [128, 9, 64], BF16, kind="ExternalInput")
    # cs = BN shifts (2 columns, needed by the first conv1 epilogue) and
    # mk = the two critic masks (only needed at quad MQ) ride the scalar
    # HWDGE ring as two separate DMAs so the tiny start-critical piece
    # lands first.
    cs_d = nc.dram_tensor("cs", [128, 2], F32, kind="ExternalInput")
    mk_d = nc.dram_tensor("mk", [64, 98], F32, kind="ExternalInput")
    o_d = nc.dram_tensor("o", [128, SLOTS, H, W], BF16, kind="ExternalOutput")

    with tile.TileContext(nc) as tc:
        with (
            tc.tile_pool(name="singles", bufs=1) as singles,
            tc.tile_pool(name="outp", bufs=4) as out_pool,
            tc.tile_pool(name="xys", bufs=1) as xy_pool,
            tc.tile_pool(name="ps1", bufs=2, space="PSUM") as ps1_pool,
            tc.tile_pool(name="ps2", bufs=2, space="PSUM") as ps2_pool,
        ):
            w1c_sb = singles.tile([128, 64], BF16, name="w1c_sb")
            w1r_sb = singles.tile([128, 8, 64], BF16, name="w1r_sb")
            w2_sb = singles.tile([128, 9, 64], BF16, name="w2_sb")
            cs_sb = singles.tile([128, 2], F32, name="cs_sb")
            mk_sb = singles.tile([128, 98], F32, name="mk_sb")
            warm_sb = singles.tile([128, 1], F32, name="warm_sb")

            def shift(i):           # [128, 1] bias column for bn shift i
                return cs_sb[:, i:i + 1]

            def mask(k):            # [64, 7, 7] critic mask k
                return mk_sb[0:64, 49 * k:49 * (k + 1)].rearrange(
                    "p (h w) -> p h w", h=H, w=W)

            def w1view(t, half):
                if t == 4:
                    return w1c_sb[64 * half:64 * half + 64, :]
                r = t if t < 4 else t - 1
                return w1r_sb[64 * half:64 * half + 64, r, :]

            def w2view(t, half):
                return w2_sb[64 * half:64 * half + 64, t, :]

            xts = [xy_pool.tile([128, 2 * NMAX, H, W], BF16,
                                name=f"xt{i}", tag=f"xt{i}")
                   for i in range(XBUF)]
            y1s = [xy_pool.tile([128, 2 * NMAX, H, W], BF16,
                                name=f"y1{i}", tag=f"y1{i}")
                   for i in range(YBUF)]

            def emit_in_dma(v):
                base, n = QUADS[v]
                nc.sync.dma_start(xts[v % XBUF][:, 0:2 * n],
                                  x_d[:, base:base + 2 * n])

            def emit_conv2(state):
                v, base, n, yp, xp = state
                # separate per-pair psum tiles: Tile deps are per-tile, so
                # a shared tile would serialize pair 0's epilogue writes
                # behind pair 1's reads (and vice versa)
                ps2 = [ps2_pool.tile([128, NMAX, H, W], F32, name=f"ps2{j}")
                       for j in range(2)]
                _emit_conv(nc, yp, w2view, ps2, n)
                if v == MQ:
                    # critic mask 2 on conv2 output of the masked image
                    tgt = ps2[0][0:64, 0, :, :]
                    nc.vector.tensor_mul(tgt, tgt, mask(1))
                # residual adds from the bf16 x tile, back-to-back on
                # DVE; the two relu(psum+shift2) finals split DVE/ACT in
                # the last quad so they run concurrently
                views = [ps2[j][:, 0:n] for j in range(2)]
                out_q = out_pool.tile([128, 2 * NMAX, H, W], BF16,
                                      name="out_q")

                def add(j):
                    nc.vector.tensor_add(
                        views[j], views[j], xp[:, n * j:n * (j + 1)])

                def relu_p1():
                    nc.scalar.activation(
                        out=out_q[:, n:2 * n], in_=views[1],
                        func=mybir.ActivationFunctionType.Relu,
                        bias=shift(1), scale=1.0)

                def ts(j):
                    nc.vector.tensor_scalar(
                        out_q[:, n * j:n * (j + 1)], views[j],
                        shift(1), 0.0,
                        mybir.AluOpType.add, mybir.AluOpType.max)

                if v >= len(QUADS) - 2:
                    # last two quads: everything after the last matmul is
                    # pure drain. Residual adds must run on DVE (Pool
                    # cannot access PSUM), but pair 1's relu+shift moves
                    # to ACT so the two pairs' conversions overlap. Each
                    # quad ships both pairs in ONE kick: a second kick
                    # costs ~0.6us of Sync-sequencer serialization that
                    # delays the final descriptors more than the earlier
                    # pair-0 start saves (measured: a split penult kick
                    # pushed the final quad's kick ~0.9us later).
                    add(0)
                    add(1)
                    ts(0)
                    relu_p1()
                    nc.sync.dma_start(o_d[:, base:base + 2 * n],
                                      out_q[:, 0:2 * n])
                else:
                    # steady state: whole epilogue on DVE (it has slack),
                    # keeping ACT free for the conv1 RELUs so the next
                    # quad's conv2 is never gated on a queued-up ACT
                    add(0)
                    add(1)
                    ts(0)
                    ts(1)
                    nc.sync.dma_start(o_d[:, base:base + 2 * n],
                                      out_q[:, 0:2 * n])

            pending = None
            for v, (base, n) in enumerate(QUADS):
                if v == 0:
                    # prologue: one dma_start spreads over all 16 sync
                    # queues, so serial kick order is the lever: quad0's
                    # x first (gates the first matmul), then w1, then the
                    # next quad, then w2 (needed one round later). The
                    # scalar ring carries the shift blob + ACT prewarm +
                    # masks in parallel with the sync kicks.
                    emit_in_dma(0)
                    nc.sync.dma_start(w1c_sb[:], w1c_d[:])
                    nc.scalar.dma_start(cs_sb[:], cs_d[:])
                    nc.scalar.memzero(warm_sb[:])
                    nc.sync.dma_start(w1r_sb[:], w1r_d[:])
                    emit_in_dma(1)
                    nc.sync.dma_start(w2_sb[:], w2_d[:])
                    nc.scalar.dma_start(mk_sb[0:64, :], mk_d[:])
                    emit_in_dma(2)
                    emit_in_dma(3)
                elif v + 3 < len(QUADS):
                    emit_in_dma(v + 3)
                xp = xts[v % XBUF]
                ps1 = [ps1_pool.tile([128, NMAX, H, W], F32, name=f"ps1{j}")
                       for j in range(2)]
                _emit_conv(nc, xp, w1view, ps1, n)
                yp = y1s[v % YBUF]
                nc.scalar.activation(
                    out=yp[:, 0:n], in_=ps1[0][:, 0:n],
                    func=mybir.ActivationFunctionType.Relu,
                    bias=shift(0), scale=1.0)
                if v == len(QUADS) - 2:
                    # penult quad: pair 1's y1 relu on DVE (idle right
                    # then) so both halves land together - the PE races
                    # through the small tail convs and conv2's pair-1
                    # chains were measured stalling ~0.4us behind a
                    # serialized second ACT relu. (The final quad keeps
                    # ACT: by then DVE is draining the last steady
                    # quad's epilogue while ACT is free.)
                    nc.vector.tensor_scalar(
                        yp[:, n:2 * n], ps1[1][:, 0:n],
                        shift(0), 0.0,
                        mybir.AluOpType.add, mybir.AluOpType.max)
                else:
                    nc.scalar.activation(
                        out=yp[:, n:2 * n], in_=ps1[1][:, 0:n],
                        func=mybir.ActivationFunctionType.Relu,
                        bias=shift(0), scale=1.0)
                if v == MQ:
                    # critic mask 1 on relu(bn1(conv1)) of the masked image
                    tgt = yp[0:64, 0, :, :]
                    nc.vector.tensor_mul(tgt, tgt, mask(0))
                if pending is not None:
                    emit_conv2(pending)
                pending = (v, base, n, yp, xp)
            emit_conv2(pending)

    nc.compile()
    return nc


def _get_nc():
    if "nc" not in _CACHE:
        _CACHE["nc"] = _build()
    return _CACHE["nc"]


def _host_pack(x, w1, g1, b1, m1, v1, w2, g2, b2, m2, v2, mask1, mask2):
    x = np.asarray(x, np.float32)
    scale1 = np.asarray(g1, np.float32) / np.sqrt(np.asarray(v1, np.float32) + EPS)
    shift1 = np.asarray(b1, np.float32) - np.asarray(m1, np.float32) * scale1
    scale2 = np.asarray(g2, np.float32) / np.sqrt(np.asarray(v2, np.float32) + EPS)
    shift2 = np.asarray(b2, np.float32) - np.asarray(m2, np.float32) * scale2

    def pack_w(w, scale):
        ws = np.asarray(w, np.float32) * scale[:, None, None, None]
        # [co, ci, kh, kw] -> [ci, tap, co], duplicated into both halves
        lhsT = ws.transpose(1, 2, 3, 0).reshape(64, 9, 64)
        return np.ascontiguousarray(np.tile(lhsT, (2, 1, 1)).astype(NP_BF16))

    wdev1, wdev2 = pack_w(w1, scale1), pack_w(w2, scale2)
    w1c = np.ascontiguousarray(wdev1[:, 4, :])
    w1r = np.ascontiguousarray(
        wdev1[:, [0, 1, 2, 3, 5, 6, 7, 8], :])
    cs = np.ascontiguousarray(
        np.tile(np.stack([shift1, shift2], 1), (2, 1)).astype(np.float32))

    def pack_mk(msk1, msk2):
        mk = np.empty((64, 98), dtype=np.float32)
        mk[:, 0:49] = np.asarray(msk1, np.float32).reshape(64, 49)
        mk[:, 49:98] = np.asarray(msk2, np.float32).reshape(64, 49)
        return np.ascontiguousarray(mk)

    # Raw (unpadded) bf16 input: [core, 128, slot, 7, 7]. Quad q (slot
    # base b, size n) holds images [g0, g0+4n) of its core: pair-major,
    # then half, then index -> partition half h holds channel block,
    # slot b + j*n + i. On core 0, batch element 0 (the masked image) is
    # swapped with position MSWAP so it lands at quad MQ's first slot.
    xb = x.reshape(NCORES, BPC, C, H, W)
    xb0 = xb[0].copy()
    xb0[[0, MSWAP]] = xb0[[MSWAP, 0]]
    xb = np.concatenate([xb0[None], xb[1:]], axis=0).astype(NP_BF16)
    xdev = np.empty((NCORES, 128, SLOTS, H, W), dtype=NP_BF16)
    g0 = 0
    for (sb, n) in QUADS:
        xq = xb[:, g0:g0 + 4 * n].reshape(NCORES, 2, 2, n, C, H, W)
        # [core, pair, half, i, c, h, w] -> [core, half, c, pair, i, h, w]
        xq = xq.transpose(0, 2, 4, 1, 3, 5, 6).reshape(
            NCORES, 128, 2 * n, H, W)
        xdev[:, :, sb:sb + 2 * n] = xq
        g0 += 4 * n

    mk0 = pack_mk(mask1, mask2)
    mk1 = np.ones((64, 98), dtype=np.float32)

    in_maps = []
    for c in range(NCORES):
        in_maps.append({
            "x": np.ascontiguousarray(xdev[c]),
            "w1c": w1c,
            "w1r": w1r,
            "w2": wdev2,
            "cs": cs,
            "mk": mk0 if c == 0 else mk1,
        })
    return in_maps


def _host_unpack(results):
    o = np.stack([results[c]["o"] for c in range(NCORES)]).astype(np.float32)
    out = np.empty((NCORES, BPC, C, H, W), dtype=np.float32)
    g0 = 0
    for (sb, n) in QUADS:
        oq = o[:, :, sb:sb + 2 * n].reshape(NCORES, 2, C, 2, n, H, W)
        # [core, half, c, pair, i, h, w] -> [core, pair, half, i, c, h, w]
        out[:, g0:g0 + 4 * n] = oq.transpose(0, 3, 1, 4, 2, 5, 6).reshape(
            NCORES, 4 * n, C, H, W)
        g0 += 4 * n
    out[0][[0, MSWAP]] = out[0][[MSWAP, 0]]
    return np.ascontiguousarray(out.reshape(B, C, H, W))


def run(trace=False, **inputs):
    nc = _get_nc()
    in_maps = _host_pack(**inputs)
    res = run_bass_kernel_spmd(nc, in_maps, core_ids=list(range(NCORES)),
                               trace=trace)
    return _host_unpack(res.results), res


def kernel(**inputs) -> np.ndarray:
    out, _ = run(trace=False, **inputs)
    return out
